# revision 1
# baseline (speedup 1.0000x reference)
"""Trainium2 Bass kernel for nn_ComplexLinearAndLeakyReLU.

Math (per batch b, point c, channel e):
  R = basis(J): rows (nU, nV, nJ);  rtx = R^T-style contraction with X;
  a,b,c fields -> Y = A@a + Bw@b + Cw@c  (contraction over e)
  then VNLeakyReLU over features: d = W@x,  x_out = x - 0.8*min(dot,0)/(dns+eps)*d.

Key reformulation (orthonormal-frame identity, exact up to O(eps)=1e-6):
  with w = (uz, 0, nz) the z-component column of R:
    a = X - w(w.X),  b = X x w,  c = w(w.X)
  which folds the 3x3 per-point rotation into 8 pointwise product planes and
  11 GEMM terms with host-precombined weights (A, Cw-A, Bw, -Bw).

Sharding: data-parallel over batch B=8 -> one batch per NeuronCore.
Host pre-transposes X,J to [3,E,C] planes so the e-contraction lands on
SBUF partitions with fully-contiguous DMA; weights are replicated.
"""

import numpy as np
from contextlib import ExitStack

import concourse.bass as bass
import concourse.tile as tile
from concourse import bacc, mybir
from concourse.bass_utils import run_bass_kernel_spmd

F32 = mybir.dt.float32
F32R = mybir.dt.float32r
ALU = mybir.AluOpType
ACTF = mybir.ActivationFunctionType

B, C, E, F = 8, 2048, 256, 256
EPS = 1e-6

# --- tunables -------------------------------------------------------------
CT = 256            # c-tile width (matmul N); C/CT tiles per core
MM_F32R = True      # fp32r matmuls (full-rate) vs plain fp32 (1/4 rate)
RECIP_FAST = True   # 1-instr approx reciprocal (~51 ULP) vs 2-instr (~2 ULP)
NBUF = dict(inp=2, tmp=1, keep=2, prod=2, xsb=2, fin=2, out=2)


def _recip(nc, pool, out, in_, tag):
    if RECIP_FAST:
        nc.vector.reciprocal_approx_fast(out=out[:], in_=in_[:])
    else:
        scratch = pool.tile(list(in_.shape), F32, tag="rscratch", name="rscratch")
        nc.vector.reciprocal_approx_accurate(out=out[:], in_=in_[:], scratch=scratch[:])


MMDT = F32R if MM_F32R else F32

def build_nc():
    nc = bacc.Bacc("TRN2", target_bir_lowering=False, debug=False, num_devices=8)

    for val in (EPS, -1.25 * EPS):
        t = nc.alloc_sbuf_tensor(f"const-f32-{val}", [128, 1], F32)
        nc.gpsimd.memset(t.ap(), val)
        nc.const_aps.aps[(F32, val)] = t.ap()
    nc.all_engine_barrier()

    xp = nc.dram_tensor("xp", [3, E, C], MMDT, kind="ExternalInput")
    jp = nc.dram_tensor("jp", [3, E, C], F32, kind="ExternalInput")
    wy = nc.dram_tensor("wy", [4, E, F], MMDT, kind="ExternalInput")  # A^T,(Cw-A)^T,Bw^T,(-Bw)^T
    wt = nc.dram_tensor("wt", [F, F], MMDT, kind="ExternalInput")     # W^T
    out = nc.dram_tensor("out", [F, 3, C], F32, kind="ExternalOutput")

    NCT = C // CT

    with tile.TileContext(nc) as tc, ExitStack() as ctx:
        wpool = ctx.enter_context(tc.tile_pool(name="w", bufs=1))
        inpool = ctx.enter_context(tc.tile_pool(name="inp", bufs=NBUF["inp"]))
        tmppool = ctx.enter_context(tc.tile_pool(name="tmp", bufs=NBUF["tmp"]))
        keeppool = ctx.enter_context(tc.tile_pool(name="keep", bufs=NBUF["keep"]))
        prodpool = ctx.enter_context(tc.tile_pool(name="prod", bufs=NBUF["prod"]))
        xsbpool = ctx.enter_context(tc.tile_pool(name="xsb", bufs=NBUF["xsb"]))
        finpool = ctx.enter_context(tc.tile_pool(name="fin", bufs=NBUF["fin"]))
        outpool = ctx.enter_context(tc.tile_pool(name="outp", bufs=NBUF["out"]))
        ypool = ctx.enter_context(tc.tile_pool(name="ypsum", bufs=1, space="PSUM"))
        dpool = ctx.enter_context(tc.tile_pool(name="dpsum", bufs=1, space="PSUM"))

        # --- weights: once, resident ---
        wy_sb = []
        for t in range(4):
            w_t = wpool.tile([128, 2, F], MMDT, tag=f"wy{t}", name=f"wy{t}")
            nc.sync.dma_start(w_t[:], wy[t].rearrange("(k p) f -> p k f", p=128))
            wy_sb.append(w_t)
        wt_sb = wpool.tile([128, 2, F], MMDT, tag="wt", name="wt")
        nc.sync.dma_start(wt_sb[:], wt.rearrange("(k p) f -> p k f", p=128))

        for ci in range(NCT):
            c0 = ci * CT

            def load(dram3, i, tag, dt=F32):
                t = inpool.tile([128, 2, CT], dt, tag=tag, name=tag)
                nc.sync.dma_start(
                    t[:], dram3[i][:, c0:c0 + CT].rearrange("(k p) c -> p k c", p=128)
                )
                return t

            jx = load(jp, 0, "jx"); jy = load(jp, 1, "jy"); jz = load(jp, 2, "jz")
            xx = load(xp, 0, "xx", MMDT); xy = load(xp, 1, "xy", MMDT); xz = load(xp, 2, "xz", MMDT)

            def T(tag, pool=None):
                return (pool or tmppool).tile([128, 2, CT], F32, tag=tag, name=tag)

            # --- basis scalars (planes over (e,c)) ---
            q1 = T("q1"); nc.scalar.square(q1[:], jx[:])
            q2 = T("q2"); nc.scalar.square(q2[:], jy[:])
            q3 = T("q3"); nc.scalar.square(q3[:], jz[:])
            t1 = T("t1"); nc.vector.tensor_add(t1[:], q1[:], q2[:])
            n2 = T("n2"); nc.gpsimd.tensor_add(n2[:], t1[:], q3[:])
            r = T("r"); nc.scalar.sqrt(r[:], n2[:])
            # D2 = jz + eps*r ;  D1r = r + eps ;  P = D1r*D2 ; i12 = 1/P
            D2 = T("D2"); nc.vector.scalar_tensor_tensor(D2[:], r[:], EPS, jz[:], ALU.mult, ALU.add)
            P = T("P"); nc.vector.scalar_tensor_tensor(P[:], r[:], EPS, D2[:], ALU.add, ALU.mult)
            i12 = T("i12"); _recip(nc, tmppool, i12, P, "i12")
            i1 = T("i1"); nc.gpsimd.tensor_mul(i1[:], D2[:], i12[:])
            nz = keeppool.tile([128, 2, CT], F32, tag="nz", name="nz")
            nc.vector.tensor_mul(nz[:], jz[:], i1[:])
            # Uz = -t1 * i12 ; s2 = t1*i1^2 ; u2 = s2 + Uz^2
            Uz = T("Uz"); nc.vector.scalar_tensor_tensor(Uz[:], t1[:], -1.0, i12[:], ALU.mult, ALU.mult)
            sqi = T("sqi"); nc.scalar.square(sqi[:], i1[:])
            s2 = T("s2"); nc.gpsimd.tensor_mul(s2[:], t1[:], sqi[:])
            uzsq = T("uzsq"); nc.scalar.square(uzsq[:], Uz[:])
            u2 = T("u2"); nc.vector.tensor_add(u2[:], s2[:], uzsq[:])
            su = T("su"); nc.scalar.sqrt(su[:], u2[:])
            D3 = T("D3"); nc.scalar.add(D3[:], su[:], EPS)
            i3 = T("i3"); _recip(nc, tmppool, i3, D3, "i3")
            uz = keeppool.tile([128, 2, CT], F32, tag="uz", name="uz")
            nc.vector.tensor_mul(uz[:], Uz[:], i3[:])

            # --- product planes ---
            al = T("al"); nc.scalar.square(al[:], uz[:])
            ga = T("ga"); nc.scalar.square(ga[:], nz[:])

            def PR(tag):
                return prodpool.tile([128, 2, CT], MMDT, tag=tag, name=tag)

            axx = PR("axx"); nc.vector.tensor_mul(axx[:], al[:], xx[:])
            gxz = PR("gxz"); nc.gpsimd.tensor_mul(gxz[:], ga[:], xz[:])
            nzxy = PR("nzxy"); nc.vector.tensor_mul(nzxy[:], nz[:], xy[:])
            uzxy = PR("uzxy"); nc.gpsimd.tensor_mul(uzxy[:], uz[:], xy[:])
            uzxz = PR("uzxz"); nc.vector.tensor_mul(uzxz[:], uz[:], xz[:])
            nzxx = PR("nzxx"); nc.gpsimd.tensor_mul(nzxx[:], nz[:], xx[:])
            bxz = PR("bxz"); nc.gpsimd.tensor_mul(bxz[:], nz[:], uzxz[:])
            bxx = PR("bxx"); nc.vector.tensor_mul(bxx[:], uz[:], nzxx[:])

            # --- Y GEMMs: 11 terms, accumulate in PSUM (all comps in one 3-bank tile) ---
            terms = {
                0: [(0, xx), (1, axx), (1, bxz), (2, nzxy)],
                1: [(0, xy), (2, uzxz), (3, nzxx)],
                2: [(0, xz), (1, bxx), (1, gxz), (3, uzxy)],
            }
            yall = ypool.tile([128, 3, 2, CT], F32, tag="yall", name="yall")
            for i in range(3):
                tl = terms[i]
                n_mm = len(tl) * 2
                for fj in range(2):
                    k = 0
                    for (tw, plane) in tl:
                        for ke in range(2):
                            nc.tensor.matmul(
                                yall[:, i, fj, :],
                                lhsT=wy_sb[tw][:, ke, fj * 128:(fj + 1) * 128],
                                rhs=plane[:, ke, :],
                                start=(k == 0), stop=(k == n_mm - 1),
                            )
                            k += 1
            xall = xsbpool.tile([128, 3, 2, CT], MMDT, tag="xall", name="xall")
            for i in range(3):
                nc.scalar.copy(xall[:, i], yall[:, i])

            # --- W GEMM (all comps into one 3-bank PSUM tile) ---
            dall = dpool.tile([128, 3, 2, CT], F32, tag="dall", name="dall")
            for i in range(3):
                for fj in range(2):
                    for kg in range(2):
                        nc.tensor.matmul(
                            dall[:, i, fj, :],
                            lhsT=wt_sb[:, kg, fj * 128:(fj + 1) * 128],
                            rhs=xall[:, i, kg, :],
                            start=(kg == 0), stop=(kg == 1),
                        )

            # --- VN-LeakyReLU tail, single instructions over all comps ---
            def FT(tag, shape=None):
                return finpool.tile(shape or [128, 2, CT], F32, tag=tag, name=tag)

            dva = FT("dva", [128, 3, 2, CT])
            nc.vector.tensor_mul(dva[:], xall[:], dall[:])
            dota = FT("fU"); nc.gpsimd.tensor_add(dota[:], dva[:, 0], dva[:, 1])
            dot = FT("fV"); nc.vector.tensor_add(dot[:], dota[:], dva[:, 2])
            ea = FT("ea", [128, 3, 2, CT])
            nc.scalar.square(ea[:], dall[:])
            dnsa = FT("fW"); nc.gpsimd.tensor_add(dnsa[:], ea[:, 0], ea[:, 1])
            dns = FT("fU"); nc.vector.tensor_add(dns[:], dnsa[:], ea[:, 2])
            # den' = (dns+eps) * -1.25 ;  inv = 1/den' = -0.8/(dns+eps)
            den = FT("fW"); nc.scalar.activation(den[:], dns[:], ACTF.Identity, bias=-1.25 * EPS, scale=-1.25)
            inv = FT("fU"); _recip(nc, finpool, inv, den, "inv")
            # rr = min(dot,0) * inv   (>= 0);  out = x + rr*d
            rr = FT("fV"); nc.vector.scalar_tensor_tensor(rr[:], dot[:], 0.0, inv[:], ALU.min, ALU.mult)
            rrap = rr[:]
            rrb = bass.AP(tensor=rrap.tensor, offset=rrap.offset,
                          ap=[rrap.ap[0], [0, 3]] + rrap.ap[1:])
            ga_ = FT("dva", [128, 3, 2, CT])
            nc.vector.tensor_mul(ga_[:], rrb, dall[:])
            oall = outpool.tile([128, 3, 2, CT], F32, tag="oall", name="oall")
            nc.vector.tensor_add(oall[:], ga_[:], xall[:])
            nc.sync.dma_start(
                out[:, :, c0:c0 + CT].rearrange("(k p) i c -> p i k c", p=128), oall[:]
            )

    nc.compile()
    return nc


_NC_CACHE = {}


def _get_nc():
    if "nc" not in _NC_CACHE:
        _NC_CACHE["nc"] = build_nc()
    return _NC_CACHE["nc"]


def kernel(X, J, A, Bw, Cw, W):
    X = np.ascontiguousarray(X, dtype=np.float32)
    J = np.ascontiguousarray(J, dtype=np.float32)
    A = np.asarray(A, dtype=np.float32)
    Bw = np.asarray(Bw, dtype=np.float32)
    Cw = np.asarray(Cw, dtype=np.float32)
    W = np.asarray(W, dtype=np.float32)

    wy = np.ascontiguousarray(
        np.stack([A.T, (Cw - A).T, Bw.T, (-Bw).T]), dtype=np.float32
    )                                   # [4, E, F]
    wt = np.ascontiguousarray(W.T)      # [F, F]

    in_maps = []
    for b in range(B):
        in_maps.append({
            "xp": np.ascontiguousarray(X[b].transpose(2, 1, 0)),  # [3,E,C]
            "jp": np.ascontiguousarray(J[b].transpose(2, 1, 0)),
            "wy": wy,
            "wt": wt,
        })

    nc = _get_nc()
    try:
        res = run_bass_kernel_spmd(nc, in_maps, core_ids=list(range(B)))
    except Exception:
        import time as _time
        _time.sleep(15)  # transient NRT device errors recover on retry
        res = run_bass_kernel_spmd(nc, in_maps, core_ids=list(range(B)))
    return np.stack([res.results[b]["out"] for b in range(B)])  # [B,F,3,C]



# revision 3
# speedup vs baseline: 1.3042x; 1.3042x over previous
"""Trainium2 Bass kernel for nn_ComplexLinearAndLeakyReLU (v2, fp16 pipeline).

Math (per batch b, point c, channel e), reformulated:
  w = (uz, 0, nz) z-column of the orthonormal frame of J, with
    nz = jz/|J|,  uz = -sign(jz+eps) * g / sqrt(t1 + g^2),  g = t1/|jz+eps|,
    t1 = jx^2+jy^2.
  s  = w . X ;  Y = A@X + (Cw-A)@[w*s] + Bw@(X x w)   (contraction over e)
  d = W@x (x = Y);  out = x - 0.8*min(dot,0)/(dns+eps) * d.

Implementation notes:
  - fp16 storage end-to-end (DVE 2x mode, matmul FWL); fp32 islands for
    range-critical basis tensors (jq, aj, g1a, gsq, w2) and inv0.
  - Engine balance: ACT does transcendental-ish single-src ops, GPS the
    small adds, DVE the tensor_tensor stream.
  - One batch per NeuronCore (8 cores), weights replicated.
"""

import numpy as np
from contextlib import ExitStack

import concourse.bass as bass
import concourse.tile as tile
from concourse import bacc, mybir
from concourse.bass_utils import run_bass_kernel_spmd

F32 = mybir.dt.float32
F16 = mybir.dt.float16
ALU = mybir.AluOpType
ACTF = mybir.ActivationFunctionType

B, C, E, F = 8, 2048, 256, 256
EPS = 1e-6

# --- tunables -------------------------------------------------------------
CW = 512            # columns per outer chunk (elementwise granularity)
NCH = C // CW
MMN = 512           # matmul moving free size (== CW)
NBUF = dict(inp=2, bas=1, prod=2, xd=2, tail=1, out=2)


def _mk(ap, dims):
    """Build an AP over the same tensor with explicit [stride, size] free dims."""
    return bass.AP(tensor=ap.tensor, offset=ap.offset, ap=[ap.ap[0]] + dims)


def build_nc():
    nc = bacc.Bacc("TRN2", target_bir_lowering=False, debug=False, num_devices=8)

    for val in (EPS, 1.25 * EPS):
        t = nc.alloc_sbuf_tensor(f"const-f32-{val}", [128, 1], F32)
        nc.gpsimd.memset(t.ap(), val)
        nc.const_aps.aps[(F32, val)] = t.ap()
    nc.all_engine_barrier()

    xp = nc.dram_tensor("xp", [3, E, C], F16, kind="ExternalInput")
    jp = nc.dram_tensor("jp", [3, E, C], F16, kind="ExternalInput")
    wy = nc.dram_tensor("wy", [4, E, F], F16, kind="ExternalInput")  # A^T,(Cw-A)^T,Bw^T,(-Bw)^T
    wt = nc.dram_tensor("wt", [F, F], F16, kind="ExternalInput")     # W^T
    out = nc.dram_tensor("out", [F, 3, C], F16, kind="ExternalOutput")

    with tile.TileContext(nc) as tc, ExitStack() as ctx:
        wpool = ctx.enter_context(tc.tile_pool(name="w", bufs=1))
        inpool = ctx.enter_context(tc.tile_pool(name="inp", bufs=NBUF["inp"]))
        baspool = ctx.enter_context(tc.tile_pool(name="bas", bufs=NBUF["bas"]))
        prodpool = ctx.enter_context(tc.tile_pool(name="prod", bufs=NBUF["prod"]))
        xdpool = ctx.enter_context(tc.tile_pool(name="xd", bufs=NBUF["xd"]))
        tailpool = ctx.enter_context(tc.tile_pool(name="tail", bufs=NBUF["tail"]))
        outpool = ctx.enter_context(tc.tile_pool(name="outp", bufs=NBUF["out"]))
        ypool = ctx.enter_context(tc.tile_pool(name="ypsum", bufs=1, space="PSUM"))
        dpool = ctx.enter_context(tc.tile_pool(name="dpsum", bufs=1, space="PSUM"))

        # --- weights: once, resident ---
        wy_sb = []
        for t in range(4):
            w_t = wpool.tile([128, 2, F], F16, tag=f"wy{t}", name=f"wy{t}")
            nc.sync.dma_start(w_t[:], wy[t].rearrange("(k p) f -> p k f", p=128))
            wy_sb.append(w_t)
        wt_sb = wpool.tile([128, 2, F], F16, tag="wt", name="wt")
        nc.sync.dma_start(wt_sb[:], wt.rearrange("(k p) f -> p k f", p=128))

        for ci in range(NCH):
            c0 = ci * CW

            J3 = inpool.tile([128, 3, 2, CW], F16, tag="J3", name="J3")
            X3 = inpool.tile([128, 3, 2, CW], F16, tag="X3", name="X3")
            for i in range(3):
                nc.sync.dma_start(
                    J3[:, i], jp[i][:, c0:c0 + CW].rearrange("(k p) c -> p k c", p=128)
                )
                nc.sync.dma_start(
                    X3[:, i], xp[i][:, c0:c0 + CW].rearrange("(k p) c -> p k c", p=128)
                )
            jz = J3[:, 2]

            def BT(tag, dt=F16, shape=None):
                return baspool.tile(shape or [128, 2, CW], dt, tag=tag, name=tag)

            # --- basis ---
            q123 = BT("q123", shape=[128, 3, 2, CW])
            nc.scalar.activation(q123[:], J3[:], ACTF.Square)
            t1 = BT("t1"); nc.vector.tensor_add(t1[:], q123[:, 0], q123[:, 1])
            n2 = BT("n2"); nc.gpsimd.tensor_add(n2[:], t1[:], q123[:, 2])
            i1 = BT("i1"); nc.scalar.activation(i1[:], n2[:], ACTF.Abs_reciprocal_sqrt)
            jq = BT("jq", F32); nc.scalar.activation(jq[:], jz, ACTF.Square, bias=EPS)
            aj = BT("aj", F32); nc.scalar.activation(aj[:], jq[:], ACTF.Abs_reciprocal_sqrt)
            sg = BT("sg"); nc.scalar.sign(sg[:], jz, bias=EPS)
            g1a = BT("g1a", F32); nc.vector.tensor_mul(g1a[:], t1[:], aj[:])
            gsq = BT("gsq", F32); nc.scalar.square(gsq[:], g1a[:])
            w2 = BT("w2", F32); nc.vector.tensor_add(w2[:], t1[:], gsq[:])
            i3 = BT("i3"); nc.scalar.activation(i3[:], w2[:], ACTF.Abs_reciprocal_sqrt)
            uzp = BT("uzp")
            nc.vector.scalar_tensor_tensor(uzp[:], g1a[:], -1.0, i3[:], ALU.mult, ALU.mult)
            # uznz: [which(uz,nz), k, c]
            uznz = baspool.tile([128, 2, 2, CW], F16, tag="uznz", name="uznz")
            nc.vector.tensor_mul(uznz[:, 0], uzp[:], sg[:])
            nc.vector.tensor_mul(uznz[:, 1], jz, i1[:])

            # --- products ---
            # UN6[w, i, kc] = uznz[w] * X3[i]
            UN6 = prodpool.tile([128, 2, 3, 2, CW], F16, tag="UN6", name="UN6")
            KC = 2 * CW
            un_in0 = _mk(uznz[:], [[KC, 2], [0, 3], [1, KC]])
            un_in1 = _mk(X3[:], [[0, 2], [KC, 3], [1, KC]])
            un_out = _mk(UN6[:], [[3 * KC, 2], [KC, 3], [1, KC]])
            nc.vector.tensor_tensor(un_out, un_in0, un_in1, ALU.mult)
            # s = uz*xx + nz*xz
            s = prodpool.tile([128, 2, CW], F16, tag="s", name="s")
            nc.gpsimd.tensor_add(s[:], UN6[:, 0, 0], UN6[:, 1, 2])
            # P02[w] = uznz[w] * s
            P02 = prodpool.tile([128, 2, 2, CW], F16, tag="P02", name="P02")
            p_in0 = _mk(uznz[:], [[KC, 2], [1, KC]])
            p_in1 = _mk(s[:], [[0, 2], [1, KC]])
            p_out = _mk(P02[:], [[KC, 2], [1, KC]])
            nc.vector.tensor_tensor(p_out, p_in0, p_in1, ALU.mult)

            xx, xy, xz = X3[:, 0], X3[:, 1], X3[:, 2]
            P0, P2 = P02[:, 0], P02[:, 1]
            Q0 = UN6[:, 1, 1]; Q2 = UN6[:, 0, 1]
            m3 = UN6[:, 0, 2]; m4 = UN6[:, 1, 0]

            # --- Y GEMMs: 9 terms ---
            terms = {
                0: [(0, xx), (1, P0), (2, Q0)],
                1: [(0, xy), (2, m3), (3, m4)],
                2: [(0, xz), (1, P2), (3, Q2)],
            }
            yall = ypool.tile([128, 3, 2, MMN], F32, tag="yall", name="yall")
            for i in range(3):
                tl = terms[i]
                n_mm = len(tl) * 2
                for fj in range(2):
                    k = 0
                    for (tw, plane) in tl:
                        for ke in range(2):
                            nc.tensor.matmul(
                                yall[:, i, fj, :],
                                lhsT=wy_sb[tw][:, ke, fj * 128:(fj + 1) * 128],
                                rhs=plane[:, ke, :],
                                start=(k == 0), stop=(k == n_mm - 1),
                            )
                            k += 1

            # xall: fp16 copy of Y for W-GEMM rhs + tail
            xall = xdpool.tile([128, 3, 2, CW], F16, tag="xall", name="xall")
            nc.vector.tensor_copy(xall[:], yall[:])

            # --- W GEMM + dsb copy, per comp ---
            dsb = xdpool.tile([128, 3, 2, CW], F16, tag="dsb", name="dsb")
            for i in range(3):
                dall = dpool.tile([128, 2, MMN], F32, tag="dall", name="dall")
                for fj in range(2):
                    for kg in range(2):
                        nc.tensor.matmul(
                            dall[:, fj, :],
                            lhsT=wt_sb[:, kg, fj * 128:(fj + 1) * 128],
                            rhs=xall[:, i, kg, :],
                            start=(kg == 0), stop=(kg == 1),
                        )
                nc.vector.tensor_copy(dsb[:, i], dall[:])

            # --- VN-LeakyReLU tail ---
            def TT(tag, dt=F16, shape=None):
                return tailpool.tile(shape or [128, 2, CW], dt, tag=tag, name=tag)

            dva = TT("dva", shape=[128, 3, 2, CW])
            nc.vector.tensor_mul(dva[:], xall[:], dsb[:])
            dot01 = TT("dot01"); nc.gpsimd.tensor_add(dot01[:], dva[:, 0], dva[:, 1])
            dot = TT("dot"); nc.gpsimd.tensor_add(dot[:], dot01[:], dva[:, 2])
            ea = TT("ea", shape=[128, 3, 2, CW])
            nc.scalar.activation(ea[:], dsb[:], ACTF.Square)
            dns01 = TT("dns01"); nc.gpsimd.tensor_add(dns01[:], ea[:, 0], ea[:, 1])
            dns = TT("dns"); nc.gpsimd.tensor_add(dns[:], dns01[:], ea[:, 2])
            # inv0 = 0.8/(dns+eps) = 1/sqrt((1.25*dns+1.25*eps)^2)
            sqW = TT("sqW", F32)
            nc.scalar.activation(sqW[:], dns[:], ACTF.Square, bias=1.25 * EPS, scale=1.25)
            inv0 = TT("inv0", F32)
            nc.scalar.activation(inv0[:], sqW[:], ACTF.Abs_reciprocal_sqrt)
            # rr = min(dot,0)*inv0  (<= 0);  out = x - (rr*d)
            rr = TT("rr")
            nc.vector.scalar_tensor_tensor(rr[:], dot[:], 0.0, inv0[:], ALU.min, ALU.mult)
            ga = TT("ga", shape=[128, 3, 2, CW])
            g_in0 = _mk(rr[:], [[0, 3], [1, KC]])
            g_in1 = _mk(dsb[:], [[KC, 3], [1, KC]])
            g_out = _mk(ga[:], [[KC, 3], [1, KC]])
            nc.vector.tensor_tensor(g_out, g_in0, g_in1, ALU.mult)
            oall = outpool.tile([128, 3, 2, CW], F16, tag="oall", name="oall")
            nc.vector.tensor_sub(oall[:], xall[:], ga[:])
            nc.sync.dma_start(
                out[:, :, c0:c0 + CW].rearrange("(k p) i c -> p i k c", p=128), oall[:]
            )

    nc.compile()
    return nc


_NC_CACHE = {}


def _get_nc():
    if "nc" not in _NC_CACHE:
        _NC_CACHE["nc"] = build_nc()
    return _NC_CACHE["nc"]


def _prep_inputs(X, J, A, Bw, Cw, W):
    A = np.asarray(A, dtype=np.float32)
    Bw = np.asarray(Bw, dtype=np.float32)
    Cw = np.asarray(Cw, dtype=np.float32)
    W = np.asarray(W, dtype=np.float32)
    wy = np.ascontiguousarray(
        np.stack([A.T, (Cw - A).T, Bw.T, (-Bw).T])
    ).astype(np.float16)                       # [4, E, F]
    wt = np.ascontiguousarray(W.T).astype(np.float16)
    in_maps = []
    for b in range(B):
        in_maps.append({
            "xp": np.ascontiguousarray(np.asarray(X[b]).transpose(2, 1, 0)).astype(np.float16),
            "jp": np.ascontiguousarray(np.asarray(J[b]).transpose(2, 1, 0)).astype(np.float16),
            "wy": wy,
            "wt": wt,
        })
    return in_maps


def kernel(X, J, A, Bw, Cw, W):
    in_maps = _prep_inputs(X, J, A, Bw, Cw, W)
    nc = _get_nc()
    try:
        res = run_bass_kernel_spmd(nc, in_maps, core_ids=list(range(B)))
    except Exception:
        import time as _time
        _time.sleep(15)  # transient NRT device errors recover on retry
        res = run_bass_kernel_spmd(nc, in_maps, core_ids=list(range(B)))
    return np.stack([np.asarray(res.results[b]["out"]).astype(np.float32) for b in range(B)])


# revision 5
# speedup vs baseline: 1.3925x; 1.0678x over previous
"""Trainium2 Bass kernel for nn_ComplexLinearAndLeakyReLU (v2, fp16 pipeline).

Math (per batch b, point c, channel e), reformulated:
  w = (uz, 0, nz) z-column of the orthonormal frame of J, with
    nz = jz/|J|,  uz = -g/sqrt(t1 + g^2),  g = t1/(jz+eps),  t1 = jx^2+jy^2.
  s  = w . X ;  Y = A@X + (Cw-A)@[w*s] + Bw@(X x w)   (contraction over e)
  d = W@x (x = Y);  out = x - 0.8*min(dot,0)/(dns+eps) * d.

Implementation notes:
  - fp16 storage end-to-end (DVE 2x mode, matmul FWL); fp32 islands for
    range-critical basis tensors (jzp, rD, g1, gsq-in-w2) and inv0.
  - dot/dns 3-way reductions run on the TENSOR engine as identity-matmul
    PSUM accumulations (frees DVE/GPS).
  - Engine balance measured from NTFF: DVE = TT stream, ACT = activations
    + dsb cast, GPS = small adds.
  - One batch per NeuronCore (8 cores), weights replicated.
"""

import numpy as np
from contextlib import ExitStack

import concourse.bass as bass
import concourse.tile as tile
from concourse import bacc, mybir
from concourse.bass_utils import run_bass_kernel_spmd

F32 = mybir.dt.float32
F16 = mybir.dt.float16
ALU = mybir.AluOpType
ACTF = mybir.ActivationFunctionType

B, C, E, F = 8, 2048, 256, 256
EPS = 1e-6

# --- tunables -------------------------------------------------------------
CW = 512            # columns per outer chunk (elementwise granularity)
NCH = C // CW
MMN = 512           # matmul moving free size (== CW)
NBUF = dict(inp=2, bas=1, prod=2, xd=2, tail=1, out=2)


def _mk(ap, dims):
    """Build an AP over the same tensor with explicit [stride, size] free dims."""
    return bass.AP(tensor=ap.tensor, offset=ap.offset, ap=[ap.ap[0]] + dims)


def build_nc():
    nc = bacc.Bacc("TRN2", target_bir_lowering=False, debug=False, num_devices=8)

    for val in (EPS, 1.25 * EPS):
        t = nc.alloc_sbuf_tensor(f"const-f32-{val}", [128, 1], F32)
        nc.gpsimd.memset(t.ap(), val)
        nc.const_aps.aps[(F32, val)] = t.ap()
    nc.all_engine_barrier()

    xp = nc.dram_tensor("xp", [3, E, C], F16, kind="ExternalInput")
    jp = nc.dram_tensor("jp", [3, E, C], F16, kind="ExternalInput")
    wy = nc.dram_tensor("wy", [4, E, F], F16, kind="ExternalInput")  # A^T,(Cw-A)^T,Bw^T,(-Bw)^T
    wt = nc.dram_tensor("wt", [F, F], F16, kind="ExternalInput")     # W^T
    ident = nc.dram_tensor("ident", [128, 128], F16, kind="ExternalInput")
    out = nc.dram_tensor("out", [F, 3, C], F16, kind="ExternalOutput")

    with tile.TileContext(nc) as tc, ExitStack() as ctx:
        wpool = ctx.enter_context(tc.tile_pool(name="w", bufs=1))
        inpool = ctx.enter_context(tc.tile_pool(name="inp", bufs=NBUF["inp"]))
        baspool = ctx.enter_context(tc.tile_pool(name="bas", bufs=NBUF["bas"]))
        prodpool = ctx.enter_context(tc.tile_pool(name="prod", bufs=NBUF["prod"]))
        xdpool = ctx.enter_context(tc.tile_pool(name="xd", bufs=NBUF["xd"]))
        tailpool = ctx.enter_context(tc.tile_pool(name="tail", bufs=NBUF["tail"]))
        outpool = ctx.enter_context(tc.tile_pool(name="outp", bufs=NBUF["out"]))
        ypool = ctx.enter_context(tc.tile_pool(name="ypsum", bufs=1, space="PSUM"))
        dpool = ctx.enter_context(tc.tile_pool(name="dpsum", bufs=1, space="PSUM"))
        rpool = ctx.enter_context(tc.tile_pool(name="rpsum", bufs=1, space="PSUM"))

        # --- weights: once, resident ---
        wy_sb = []
        for t in range(4):
            w_t = wpool.tile([128, 2, F], F16, tag=f"wy{t}", name=f"wy{t}")
            nc.sync.dma_start(w_t[:], wy[t].rearrange("(k p) f -> p k f", p=128))
            wy_sb.append(w_t)
        wt_sb = wpool.tile([128, 2, F], F16, tag="wt", name="wt")
        nc.sync.dma_start(wt_sb[:], wt.rearrange("(k p) f -> p k f", p=128))
        id_sb = wpool.tile([128, 128], F16, tag="ident", name="ident")
        nc.sync.dma_start(id_sb[:], ident[:, :])

        for ci in range(NCH):
            c0 = ci * CW

            J3 = inpool.tile([128, 3, 2, CW], F16, tag="J3", name="J3")
            X3 = inpool.tile([128, 3, 2, CW], F16, tag="X3", name="X3")
            for i in range(3):
                nc.sync.dma_start(
                    J3[:, i], jp[i][:, c0:c0 + CW].rearrange("(k p) c -> p k c", p=128)
                )
                nc.sync.dma_start(
                    X3[:, i], xp[i][:, c0:c0 + CW].rearrange("(k p) c -> p k c", p=128)
                )
            jz = J3[:, 2]

            def BT(tag, dt=F16, shape=None):
                return baspool.tile(shape or [128, 2, CW], dt, tag=tag, name=tag)

            # --- basis ---
            q123 = BT("q123", shape=[128, 3, 2, CW])
            nc.scalar.activation(q123[:], J3[:], ACTF.Square)
            t1 = BT("t1"); nc.gpsimd.tensor_add(t1[:], q123[:, 0], q123[:, 1])
            n2 = BT("n2"); nc.gpsimd.tensor_add(n2[:], t1[:], q123[:, 2])
            i1 = BT("i1"); nc.scalar.activation(i1[:], n2[:], ACTF.Abs_reciprocal_sqrt)
            jzp = BT("jzp", F32)
            nc.scalar.activation(jzp[:], jz, ACTF.Identity, bias=EPS)
            rD = BT("rD", F32); nc.vector.reciprocal_approx_fast(rD[:], jzp[:])
            g1 = BT("g1", F32); nc.vector.tensor_mul(g1[:], t1[:], rD[:])
            gsq = BT("gsq", F32); nc.scalar.square(gsq[:], g1[:])
            w2 = BT("w2", F32); nc.gpsimd.tensor_add(w2[:], t1[:], gsq[:])
            i3 = BT("i3"); nc.scalar.activation(i3[:], w2[:], ACTF.Abs_reciprocal_sqrt)
            # uznz: [which(uz,nz), k, c]
            uznz = baspool.tile([128, 2, 2, CW], F16, tag="uznz", name="uznz")
            nc.vector.scalar_tensor_tensor(uznz[:, 0], g1[:], -1.0, i3[:], ALU.mult, ALU.mult)
            nc.vector.tensor_mul(uznz[:, 1], jz, i1[:])

            # --- products ---
            # UN6[w, i, kc] = uznz[w] * X3[i]
            UN6 = prodpool.tile([128, 2, 3, 2, CW], F16, tag="UN6", name="UN6")
            KC = 2 * CW
            un_in0 = _mk(uznz[:], [[KC, 2], [0, 3], [1, KC]])
            un_in1 = _mk(X3[:], [[0, 2], [KC, 3], [1, KC]])
            un_out = _mk(UN6[:], [[3 * KC, 2], [KC, 3], [1, KC]])
            nc.vector.tensor_tensor(un_out, un_in0, un_in1, ALU.mult)
            # s = uz*xx + nz*xz
            s = prodpool.tile([128, 2, CW], F16, tag="s", name="s")
            nc.gpsimd.tensor_add(s[:], UN6[:, 0, 0], UN6[:, 1, 2])
            # P02[w] = uznz[w] * s
            P02 = prodpool.tile([128, 2, 2, CW], F16, tag="P02", name="P02")
            p_in0 = _mk(uznz[:], [[KC, 2], [1, KC]])
            p_in1 = _mk(s[:], [[0, 2], [1, KC]])
            p_out = _mk(P02[:], [[KC, 2], [1, KC]])
            nc.vector.tensor_tensor(p_out, p_in0, p_in1, ALU.mult)

            xx, xy, xz = X3[:, 0], X3[:, 1], X3[:, 2]
            P0, P2 = P02[:, 0], P02[:, 1]
            Q0 = UN6[:, 1, 1]; Q2 = UN6[:, 0, 1]
            m3 = UN6[:, 0, 2]; m4 = UN6[:, 1, 0]

            # --- per-comp: Y GEMM -> xall cast -> W GEMM -> dsb cast ---
            terms = {
                0: [(0, xx), (1, P0), (2, Q0)],
                1: [(0, xy), (2, m3), (3, m4)],
                2: [(0, xz), (1, P2), (3, Q2)],
            }
            xall = xdpool.tile([128, 3, 2, CW], F16, tag="xall", name="xall")
            dsb = xdpool.tile([128, 3, 2, CW], F16, tag="dsb", name="dsb")
            for i in range(3):
                yall = ypool.tile([128, 2, MMN], F32, tag="yall", name="yall")
                tl = terms[i]
                n_mm = len(tl) * 2
                for fj in range(2):
                    k = 0
                    for (tw, plane) in tl:
                        for ke in range(2):
                            nc.tensor.matmul(
                                yall[:, fj, :],
                                lhsT=wy_sb[tw][:, ke, fj * 128:(fj + 1) * 128],
                                rhs=plane[:, ke, :],
                                start=(k == 0), stop=(k == n_mm - 1),
                            )
                            k += 1
                nc.vector.tensor_copy(xall[:, i], yall[:])
                dall = dpool.tile([128, 2, MMN], F32, tag="dall", name="dall")
                for fj in range(2):
                    for kg in range(2):
                        nc.tensor.matmul(
                            dall[:, fj, :],
                            lhsT=wt_sb[:, kg, fj * 128:(fj + 1) * 128],
                            rhs=xall[:, i, kg, :],
                            start=(kg == 0), stop=(kg == 1),
                        )
                nc.scalar.copy(dsb[:, i], dall[:])

            # --- VN-LeakyReLU tail ---
            def TT(tag, dt=F16, shape=None):
                return tailpool.tile(shape or [128, 2, CW], dt, tag=tag, name=tag)

            dva = TT("dva", shape=[128, 3, 2, CW])
            nc.vector.tensor_mul(dva[:], xall[:], dsb[:])
            ea = TT("ea", shape=[128, 3, 2, CW])
            nc.vector.tensor_mul(ea[:], dsb[:], dsb[:])
            # dot/dns: 3-way reductions as identity-matmul PSUM accumulation
            dotP = rpool.tile([128, 2 * CW], F32, tag="dotP", name="dotP")
            dnsP = rpool.tile([128, 2 * CW], F32, tag="dnsP", name="dnsP")
            for fj in range(2):
                for i in range(3):
                    nc.tensor.matmul(
                        dotP[:, fj * CW:(fj + 1) * CW], lhsT=id_sb[:],
                        rhs=dva[:, i, fj, :], start=(i == 0), stop=(i == 2),
                    )
            for fj in range(2):
                for i in range(3):
                    nc.tensor.matmul(
                        dnsP[:, fj * CW:(fj + 1) * CW], lhsT=id_sb[:],
                        rhs=ea[:, i, fj, :], start=(i == 0), stop=(i == 2),
                    )
            # inv0 = 0.8/(dns+eps) = 1/sqrt((1.25*dns+1.25*eps)^2)
            sqW = TT("sqW", F32)
            nc.scalar.activation(sqW[:], _mk(dnsP[:], [[CW, 2], [1, CW]]),
                                 ACTF.Square, bias=1.25 * EPS, scale=1.25)
            inv0 = TT("inv0", F32)
            nc.scalar.activation(inv0[:], sqW[:], ACTF.Abs_reciprocal_sqrt)
            # rr = min(dot,0)*inv0  (<= 0);  out = x - (rr*d)
            rr = TT("rr")
            nc.vector.scalar_tensor_tensor(
                rr[:], _mk(dotP[:], [[CW, 2], [1, CW]]), 0.0, inv0[:], ALU.min, ALU.mult
            )
            ga = TT("ga", shape=[128, 3, 2, CW])
            g_in0 = _mk(rr[:], [[0, 3], [1, KC]])
            g_in1 = _mk(dsb[:], [[KC, 3], [1, KC]])
            g_out = _mk(ga[:], [[KC, 3], [1, KC]])
            nc.vector.tensor_tensor(g_out, g_in0, g_in1, ALU.mult)
            oall = outpool.tile([128, 3, 2, CW], F16, tag="oall", name="oall")
            nc.vector.tensor_sub(oall[:], xall[:], ga[:])
            nc.sync.dma_start(
                out[:, :, c0:c0 + CW].rearrange("(k p) i c -> p i k c", p=128), oall[:]
            )

    nc.compile()
    return nc


_NC_CACHE = {}


def _get_nc():
    if "nc" not in _NC_CACHE:
        _NC_CACHE["nc"] = build_nc()
    return _NC_CACHE["nc"]


def _prep_inputs(X, J, A, Bw, Cw, W):
    A = np.asarray(A, dtype=np.float32)
    Bw = np.asarray(Bw, dtype=np.float32)
    Cw = np.asarray(Cw, dtype=np.float32)
    W = np.asarray(W, dtype=np.float32)
    wy = np.ascontiguousarray(
        np.stack([A.T, (Cw - A).T, Bw.T, (-Bw).T])
    ).astype(np.float16)                       # [4, E, F]
    wt = np.ascontiguousarray(W.T).astype(np.float16)
    ident = np.eye(128, dtype=np.float16)
    in_maps = []
    for b in range(B):
        in_maps.append({
            "xp": np.ascontiguousarray(np.asarray(X[b]).transpose(2, 1, 0)).astype(np.float16),
            "jp": np.ascontiguousarray(np.asarray(J[b]).transpose(2, 1, 0)).astype(np.float16),
            "wy": wy,
            "wt": wt,
            "ident": ident,
        })
    return in_maps


def kernel(X, J, A, Bw, Cw, W):
    in_maps = _prep_inputs(X, J, A, Bw, Cw, W)
    nc = _get_nc()
    try:
        res = run_bass_kernel_spmd(nc, in_maps, core_ids=list(range(B)))
    except Exception:
        import time as _time
        _time.sleep(15)  # transient NRT device errors recover on retry
        res = run_bass_kernel_spmd(nc, in_maps, core_ids=list(range(B)))
    return np.stack([np.asarray(res.results[b]["out"]).astype(np.float32) for b in range(B)])


# revision 9
# speedup vs baseline: 1.5981x; 1.1476x over previous
"""Trainium2 Bass kernel for nn_ComplexLinearAndLeakyReLU (v2, fp16 pipeline).

Math (per batch b, point c, channel e), reformulated:
  w = (uz, 0, nz) z-column of the orthonormal frame of J, with
    nz = jz/|J|,  uz = -g/sqrt(t1 + g^2),  g = t1/(jz+eps),  t1 = jx^2+jy^2.
  s  = w . X ;  Y = A@X + (Cw-A)@[w*s] + Bw@(X x w)   (contraction over e)
  d = W@x (x = Y);  out = x - 0.8*min(dot,0)/(dns+eps) * d.

Implementation notes:
  - fp16 storage end-to-end (DVE 2x mode, matmul FWL); fp32 islands for
    range-critical basis tensors (jzp, rD, g1, gsq-in-w2) and inv0.
  - dot/dns 3-way reductions run on the TENSOR engine as identity-matmul
    PSUM accumulations (frees DVE/GPS).
  - Engine balance measured from NTFF: DVE = TT stream, ACT = activations
    + dsb cast, GPS = small adds.
  - One batch per NeuronCore (8 cores), weights replicated.
"""

import numpy as np
from contextlib import ExitStack

import concourse.bass as bass
import concourse.tile as tile
from concourse import bacc, mybir
from concourse.bass_utils import run_bass_kernel_spmd

F32 = mybir.dt.float32
F16 = mybir.dt.float16
ALU = mybir.AluOpType
ACTF = mybir.ActivationFunctionType

B, C, E, F = 8, 2048, 256, 256
EPS = 1e-6

# --- tunables -------------------------------------------------------------
CW = 512            # columns per outer chunk (elementwise granularity)
NCH = C // CW
MMN = 512           # matmul moving free size (== CW)
NBUF = dict(inp=2, bas=2, prod=2, xd=2, tail=1, out=2)


def _mk(ap, dims):
    """Build an AP over the same tensor with explicit [stride, size] free dims."""
    return bass.AP(tensor=ap.tensor, offset=ap.offset, ap=[ap.ap[0]] + dims)


def build_nc():
    nc = bacc.Bacc("TRN2", target_bir_lowering=False, debug=False, num_devices=8)

    for val in (EPS, 1.25 * EPS):
        t = nc.alloc_sbuf_tensor(f"const-f32-{val}", [128, 1], F32)
        nc.gpsimd.memset(t.ap(), val)
        nc.const_aps.aps[(F32, val)] = t.ap()
    nc.all_engine_barrier()

    xp = nc.dram_tensor("xp", [3, E, C], F16, kind="ExternalInput")
    jp = nc.dram_tensor("jp", [3, E, C], F16, kind="ExternalInput")
    wy = nc.dram_tensor("wy", [4, E, F], F16, kind="ExternalInput")  # A^T,(Cw-A)^T,Bw^T,(-Bw)^T
    wt = nc.dram_tensor("wt", [F, F], F16, kind="ExternalInput")     # W^T
    ident = nc.dram_tensor("ident", [128, 128], F16, kind="ExternalInput")
    out = nc.dram_tensor("out", [F, 3, C], F16, kind="ExternalOutput")

    with tile.TileContext(nc) as tc, ExitStack() as ctx:
        wpool = ctx.enter_context(tc.tile_pool(name="w", bufs=1))
        inpool = ctx.enter_context(tc.tile_pool(name="inp", bufs=NBUF["inp"]))
        baspool = ctx.enter_context(tc.tile_pool(name="bas", bufs=NBUF["bas"]))
        prodpool = ctx.enter_context(tc.tile_pool(name="prod", bufs=NBUF["prod"]))
        xdpool = ctx.enter_context(tc.tile_pool(name="xd", bufs=NBUF["xd"]))
        tailpool = ctx.enter_context(tc.tile_pool(name="tail", bufs=NBUF["tail"]))
        outpool = ctx.enter_context(tc.tile_pool(name="outp", bufs=NBUF["out"]))
        ypool = ctx.enter_context(tc.tile_pool(name="ypsum", bufs=2, space="PSUM"))
        dpool = ctx.enter_context(tc.tile_pool(name="dpsum", bufs=2, space="PSUM"))
        rpool = dpool  # dall / dotP / dnsP rotate through the same 2x2-bank slots

        # --- weights: once, resident ---
        wy_sb = []
        for t in range(4):
            w_t = wpool.tile([128, 2, F], F16, tag=f"wy{t}", name=f"wy{t}")
            nc.sync.dma_start(w_t[:], wy[t].rearrange("(k p) f -> p k f", p=128))
            wy_sb.append(w_t)
        wt_sb = wpool.tile([128, 2, F], F16, tag="wt", name="wt")
        nc.sync.dma_start(wt_sb[:], wt.rearrange("(k p) f -> p k f", p=128))
        id_sb = wpool.tile([128, 128], F16, tag="ident", name="ident")
        nc.sync.dma_start(id_sb[:], ident[:, :])

        for ci in range(NCH):
            c0 = ci * CW

            J3 = inpool.tile([128, 3, 2, CW], F16, tag="J3", name="J3")
            X3 = inpool.tile([128, 3, 2, CW], F16, tag="X3", name="X3")
            for i in range(3):
                nc.sync.dma_start(
                    J3[:, i], jp[i][:, c0:c0 + CW].rearrange("(k p) c -> p k c", p=128)
                )
                nc.sync.dma_start(
                    X3[:, i], xp[i][:, c0:c0 + CW].rearrange("(k p) c -> p k c", p=128)
                )
            jz = J3[:, 2]

            def BT(tag, dt=F16, shape=None):
                return baspool.tile(shape or [128, 2, CW], dt, tag=tag, name=tag)

            # --- basis ---
            q123 = BT("q123", shape=[128, 3, 2, CW])
            nc.scalar.activation(q123[:], J3[:], ACTF.Square)
            t1 = BT("t1"); nc.gpsimd.tensor_add(t1[:], q123[:, 0], q123[:, 1])
            n2 = BT("n2"); nc.gpsimd.tensor_add(n2[:], t1[:], q123[:, 2])
            i1 = BT("i1"); nc.scalar.activation(i1[:], n2[:], ACTF.Abs_reciprocal_sqrt)
            jzp = BT("jzp", F32)
            nc.scalar.activation(jzp[:], jz, ACTF.Identity, bias=EPS)
            rD = BT("rD", F32); nc.vector.reciprocal_approx_fast(rD[:], jzp[:])
            g1 = BT("g1", F32); nc.vector.tensor_mul(g1[:], t1[:], rD[:])
            gsq = BT("gsq", F32); nc.scalar.square(gsq[:], g1[:])
            w2 = BT("w2", F32); nc.gpsimd.tensor_add(w2[:], t1[:], gsq[:])
            i3 = BT("i3"); nc.scalar.activation(i3[:], w2[:], ACTF.Abs_reciprocal_sqrt)
            # uznz: [which(uz,nz), k, c]
            uznz = baspool.tile([128, 2, 2, CW], F16, tag="uznz", name="uznz")
            nc.vector.scalar_tensor_tensor(uznz[:, 0], g1[:], -1.0, i3[:], ALU.mult, ALU.mult)
            nc.vector.tensor_mul(uznz[:, 1], jz, i1[:])

            # --- products ---
            # UN6[w, i, kc] = uznz[w] * X3[i]
            UN6 = prodpool.tile([128, 2, 3, 2, CW], F16, tag="UN6", name="UN6")
            KC = 2 * CW
            un_in0 = _mk(uznz[:], [[KC, 2], [0, 3], [1, KC]])
            un_in1 = _mk(X3[:], [[0, 2], [KC, 3], [1, KC]])
            un_out = _mk(UN6[:], [[3 * KC, 2], [KC, 3], [1, KC]])
            nc.vector.tensor_tensor(un_out, un_in0, un_in1, ALU.mult)
            # s = uz*xx + nz*xz
            s = prodpool.tile([128, 2, CW], F16, tag="s", name="s")
            nc.gpsimd.tensor_add(s[:], UN6[:, 0, 0], UN6[:, 1, 2])
            # P02[w] = uznz[w] * s
            P02 = prodpool.tile([128, 2, 2, CW], F16, tag="P02", name="P02")
            p_in0 = _mk(uznz[:], [[KC, 2], [1, KC]])
            p_in1 = _mk(s[:], [[0, 2], [1, KC]])
            p_out = _mk(P02[:], [[KC, 2], [1, KC]])
            nc.vector.tensor_tensor(p_out, p_in0, p_in1, ALU.mult)

            xx, xy, xz = X3[:, 0], X3[:, 1], X3[:, 2]
            P0, P2 = P02[:, 0], P02[:, 1]
            Q0 = UN6[:, 1, 1]; Q2 = UN6[:, 0, 1]
            m3 = UN6[:, 0, 2]; m4 = UN6[:, 1, 0]

            # --- per-comp: Y GEMM -> xall cast -> W GEMM -> dsb cast ---
            terms = {
                0: [(0, xx), (1, P0), (2, Q0)],
                1: [(0, xy), (2, m3), (3, m4)],
                2: [(0, xz), (1, P2), (3, Q2)],
            }
            xall = xdpool.tile([128, 3, 2, CW], F16, tag="xall", name="xall")
            dsb = xdpool.tile([128, 3, 2, CW], F16, tag="dsb", name="dsb")
            for i in range(3):
                yall = ypool.tile([128, 2, MMN], F32, tag="yall", name="yall")
                tl = terms[i]
                n_mm = len(tl) * 2
                for fj in range(2):
                    k = 0
                    for (tw, plane) in tl:
                        for ke in range(2):
                            nc.tensor.matmul(
                                yall[:, fj, :],
                                lhsT=wy_sb[tw][:, ke, fj * 128:(fj + 1) * 128],
                                rhs=plane[:, ke, :],
                                start=(k == 0), stop=(k == n_mm - 1),
                            )
                            k += 1
                nc.vector.tensor_copy(xall[:, i], yall[:])
                dall = dpool.tile([128, 2, MMN], F32, tag="dr", name="dall")
                for fj in range(2):
                    for kg in range(2):
                        nc.tensor.matmul(
                            dall[:, fj, :],
                            lhsT=wt_sb[:, kg, fj * 128:(fj + 1) * 128],
                            rhs=xall[:, i, kg, :],
                            start=(kg == 0), stop=(kg == 1),
                        )
                nc.scalar.copy(dsb[:, i], dall[:])

            # --- VN-LeakyReLU tail ---
            def TT(tag, dt=F16, shape=None):
                return tailpool.tile(shape or [128, 2, CW], dt, tag=tag, name=tag)

            dva = TT("dva", shape=[128, 3, 2, CW])
            nc.vector.tensor_mul(dva[:], xall[:], dsb[:])
            ea = TT("ea", shape=[128, 3, 2, CW])
            nc.vector.tensor_mul(ea[:], dsb[:], dsb[:])
            # dot/dns: 3-way reductions as identity-matmul PSUM accumulation
            dotP = rpool.tile([128, 2 * CW], F32, tag="dr", name="dotP")
            dnsP = rpool.tile([128, 2 * CW], F32, tag="dr", name="dnsP")
            for fj in range(2):
                for i in range(3):
                    nc.tensor.matmul(
                        dotP[:, fj * CW:(fj + 1) * CW], lhsT=id_sb[:],
                        rhs=dva[:, i, fj, :], start=(i == 0), stop=(i == 2),
                    )
            for fj in range(2):
                for i in range(3):
                    nc.tensor.matmul(
                        dnsP[:, fj * CW:(fj + 1) * CW], lhsT=id_sb[:],
                        rhs=ea[:, i, fj, :], start=(i == 0), stop=(i == 2),
                    )
            # inv0 = 0.8/(dns+eps) = 1/sqrt((1.25*dns+1.25*eps)^2)
            sqW = TT("sqW", F32)
            nc.scalar.activation(sqW[:], _mk(dnsP[:], [[CW, 2], [1, CW]]),
                                 ACTF.Square, bias=1.25 * EPS, scale=1.25)
            inv0 = TT("inv0", F32)
            nc.scalar.activation(inv0[:], sqW[:], ACTF.Abs_reciprocal_sqrt)
            # rr = min(dot,0)*inv0  (<= 0);  out = x - (rr*d)
            rr = TT("rr")
            nc.vector.scalar_tensor_tensor(
                rr[:], _mk(dotP[:], [[CW, 2], [1, CW]]), 0.0, inv0[:], ALU.min, ALU.mult
            )
            ga = TT("ga", shape=[128, 3, 2, CW])
            g_in0 = _mk(rr[:], [[0, 3], [1, KC]])
            g_in1 = _mk(dsb[:], [[KC, 3], [1, KC]])
            g_out = _mk(ga[:], [[KC, 3], [1, KC]])
            nc.vector.tensor_tensor(g_out, g_in0, g_in1, ALU.mult)
            oall = outpool.tile([128, 3, 2, CW], F16, tag="oall", name="oall")
            nc.vector.tensor_sub(oall[:], xall[:], ga[:])
            nc.sync.dma_start(
                out[:, :, c0:c0 + CW].rearrange("(k p) i c -> p i k c", p=128), oall[:]
            )

    nc.compile()
    return nc


_NC_CACHE = {}


def _get_nc():
    if "nc" not in _NC_CACHE:
        _NC_CACHE["nc"] = build_nc()
    return _NC_CACHE["nc"]


def _prep_inputs(X, J, A, Bw, Cw, W):
    A = np.asarray(A, dtype=np.float32)
    Bw = np.asarray(Bw, dtype=np.float32)
    Cw = np.asarray(Cw, dtype=np.float32)
    W = np.asarray(W, dtype=np.float32)
    wy = np.ascontiguousarray(
        np.stack([A.T, (Cw - A).T, Bw.T, (-Bw).T])
    ).astype(np.float16)                       # [4, E, F]
    wt = np.ascontiguousarray(W.T).astype(np.float16)
    ident = np.eye(128, dtype=np.float16)
    in_maps = []
    for b in range(B):
        in_maps.append({
            "xp": np.ascontiguousarray(np.asarray(X[b]).transpose(2, 1, 0)).astype(np.float16),
            "jp": np.ascontiguousarray(np.asarray(J[b]).transpose(2, 1, 0)).astype(np.float16),
            "wy": wy,
            "wt": wt,
            "ident": ident,
        })
    return in_maps


def kernel(X, J, A, Bw, Cw, W):
    in_maps = _prep_inputs(X, J, A, Bw, Cw, W)
    nc = _get_nc()
    try:
        res = run_bass_kernel_spmd(nc, in_maps, core_ids=list(range(B)))
    except Exception:
        import time as _time
        _time.sleep(15)  # transient NRT device errors recover on retry
        res = run_bass_kernel_spmd(nc, in_maps, core_ids=list(range(B)))
    return np.stack([np.asarray(res.results[b]["out"]).astype(np.float32) for b in range(B)])
